# revision 12
# baseline (speedup 1.0000x reference)
"""AugmentedLstm Trainium2 kernel.

Math (faithful to the reference):
  pi = x_t @ Wt + b ; ps = h @ Wt + b   (Wt = W_in.T, [D, 6H])
  g  = pi[:, :5H] + ps[:, :5H] = (x_t + h) @ W5 + 2*b5      (W5 = Wt[:, :5H])
  gates i,f,m,o,hw from g;  c = i*m + f*c ; out = o*tanh(c)
  h = hw*out + (1-hw)*pi6   with pi6 = x_t @ W6 + b6 (precomputable, batched)
Masking (out/mem zeroed past the sequence length) only affects outputs at
t >= len, which we zero on the host; outputs for t < len are bit-identical.

Device strategy (8 cores, data-parallel, 2 sequences/core):
  - pi6 precomputed for all t with a PE-efficient batched matmul -> DRAM.
  - Serial step loop: per step one W-stationary matmul set (80 matmuls of
    [128k x 128m] bf16 weight tiles vs [128, 2] rhs = u.T = (x_t+h).T),
    output lands H-on-partitions for cheap [128, 8]-shaped gate math.
  - All layouts are "T-orientation": partitions = h-position within a
    128-chunk, free cols = (chunk k in 4, batch b in 2).
"""

import numpy as np
import ml_dtypes

H = 512
NG = 5          # gates
M5 = 20         # 5H / 128 m-chunks
KC = 4          # 512 / 128 k-chunks
BL = 2          # sequences per core
NCORES = 8
CW = BL * KC    # columns per step slice (= 8)

_CACHE = {}


def _build(T, R, iters=None):
    import concourse.bass as bass
    import concourse.mybir as mybir
    import concourse.tile as tile
    from concourse import bacc
    from concourse.bass import ds

    f32 = mybir.dt.float32
    bf16 = mybir.dt.bfloat16
    AF = mybir.ActivationFunctionType
    ALU = mybir.AluOpType

    CH = min(256, T)
    assert T % CH == 0 and T % R == 0

    nc = bacc.Bacc("TRN2", target_bir_lowering=False, debug=False,
                   num_devices=NCORES)
    xT = nc.dram_tensor("xT", [128, (T + R) * CW], f32, kind="ExternalInput")
    w5 = nc.dram_tensor("w5", [128, M5 * KC * 128], bf16, kind="ExternalInput")
    w6 = nc.dram_tensor("w6", [128, KC * KC * 128], f32, kind="ExternalInput")
    b5k = nc.dram_tensor("b5k", [M5, 128], bf16, kind="ExternalInput")
    sel = nc.dram_tensor("sel", [KC, CW], bf16, kind="ExternalInput")
    b6 = nc.dram_tensor("b6", [128, KC], f32, kind="ExternalInput")
    outT = nc.dram_tensor("outT", [128, T * CW], f32, kind="ExternalOutput")

    with tile.TileContext(nc) as tc:
        with (
            tc.tile_pool(name="const", bufs=1) as constp,
            tc.tile_pool(name="dram", bufs=1, space="DRAM") as dramp,
            tc.tile_pool(name="state", bufs=1) as statep,
        ):
            w5_sb = constp.tile([128, M5, KC, 128], bf16)
            nc.sync.dma_start(w5_sb[:], w5[:])
            w6_sb = constp.tile([128, KC, KC, 128], f32)
            nc.sync.dma_start(w6_sb[:], w6[:])
            b5k_sb = []
            for G in range(NG):
                t_b5k = constp.tile([KC, 128], bf16, name=f"b5k{G}")
                nc.sync.dma_start(t_b5k[:], b5k[KC * G:KC * G + KC, :])
                b5k_sb.append(t_b5k)
            sel_sb = constp.tile([KC, CW], bf16)
            nc.sync.dma_start(sel_sb[:], sel[:])
            b6_sb = constp.tile([128, KC], f32)
            nc.sync.dma_start(b6_sb[:], b6[:])

            pi6T = dramp.tile([128, T * CW], f32)

            # ---- Phase B: pi6 = x @ W6 + b6 for all t, batched over time ----
            with (
                tc.tile_pool(name="bx", bufs=2) as bxp,
                tc.tile_pool(name="bo", bufs=2) as bop,
                tc.tile_pool(name="bps", bufs=4, space="PSUM") as bpsp,
            ):
                for ct in range(T // CH):
                    xc = bxp.tile([128, CH, KC, BL], f32)
                    nc.sync.dma_start(xc[:], xT[:, ct * CH * CW:(ct + 1) * CH * CW])
                    ob = bop.tile([128, CH, KC, BL], f32)
                    for m in range(KC):
                        ps = bpsp.tile([128, CH, BL], f32, tag="bps")
                        for k in range(KC):
                            nc.tensor.matmul(ps[:], w6_sb[:, m, k, :],
                                             xc[:, :, k, :],
                                             start=(k == 0), stop=(k == KC - 1))
                        nc.scalar.activation(ob[:, :, m, :], ps[:], AF.Identity,
                                             bias=b6_sb[:, m:m + 1])
                    nc.sync.dma_start(pi6T[:, ct * CH * CW:(ct + 1) * CH * CW],
                                      ob[:])

            # ---- Phase C: the serial recurrence ----
            c_st = statep.tile([128, KC, BL], f32)
            nc.vector.memset(c_st[:], 0.0)
            u_st = statep.tile([128, KC, BL], bf16)
            x0 = statep.tile([128, KC, BL], f32)
            nc.sync.dma_start(x0[:], xT[:, 0:CW])
            nc.vector.tensor_copy(u_st[:], x0[:])  # h0 = 0 -> u0 = x0

            with (
                tc.tile_pool(name="cx", bufs=2) as cxp,
                tc.tile_pool(name="cp", bufs=2) as cpp,
                tc.tile_pool(name="cr", bufs=2) as crp,
                tc.tile_pool(name="cs", bufs=2) as csp,
                tc.tile_pool(name="cu", bufs=2) as cup,
                tc.tile_pool(name="cps", bufs=1, space="PSUM") as cpsp,
            ):
                with tc.For_i(0, (T // R) if iters is None else iters) as i:
                    xblk = cxp.tile([128, R, KC, BL], f32)
                    nc.sync.dma_start(xblk[:], xT[:, ds((i * R + 1) * CW, R * CW)])
                    pblk = cpp.tile([128, R, KC, BL], f32)
                    nc.sync.dma_start(pblk[:], pi6T[:, ds(i * R * CW, R * CW)])
                    ring = crp.tile([128, R, KC, BL], f32)
                    xp = cxp.tile([128, R, KC, BL], f32, tag="xp")
                    nc.vector.tensor_add(xp[:], xblk[:], pblk[:])

                    u_cur = u_st
                    for s in range(R):
                        psg = [cpsp.tile([128, KC, BL], f32, tag=f"g{G}",
                                         name=f"psg{G}")
                               for G in range(NG)]
                        for G in range(NG):
                            nc.tensor.matmul(
                                psg[G][:], b5k_sb[G][:],
                                sel_sb[:], start=True, stop=False,
                                skip_group_check=True)
                            for ms in range(KC):
                                m = KC * G + ms
                                for k in range(KC):
                                    nc.tensor.matmul(
                                        psg[G][:, ms, :], w5_sb[:, m, k, :],
                                        u_cur[:, k, :],
                                        start=False, stop=(k == KC - 1),
                                        skip_group_check=True)
                        ig = csp.tile([128, KC, BL], f32, tag="ig")
                        fg = csp.tile([128, KC, BL], f32, tag="fg")
                        mg = csp.tile([128, KC, BL], f32, tag="mg")
                        og = csp.tile([128, KC, BL], f32, tag="og")
                        hg = csp.tile([128, KC, BL], f32, tag="hg")
                        nc.scalar.activation(ig[:], psg[0][:], AF.Sigmoid)
                        nc.scalar.activation(fg[:], psg[1][:], AF.Sigmoid)
                        nc.scalar.activation(mg[:], psg[2][:], AF.Tanh)
                        nc.scalar.activation(og[:], psg[3][:], AF.Sigmoid)
                        nc.scalar.activation(hg[:], psg[4][:], AF.Sigmoid)
                        t1 = csp.tile([128, KC, BL], f32, tag="t1")
                        nc.vector.tensor_mul(t1[:], ig[:], mg[:])
                        t2 = csp.tile([128, KC, BL], f32, tag="t2")
                        nc.vector.tensor_mul(t2[:], fg[:], c_st[:])
                        nc.vector.tensor_add(c_st[:], t1[:], t2[:])
                        tch = csp.tile([128, KC, BL], f32, tag="tch")
                        nc.scalar.activation(tch[:], c_st[:], AF.Tanh)
                        opv = csp.tile([128, KC, BL], f32, tag="opv")
                        nc.vector.tensor_mul(opv[:], og[:], tch[:])
                        dv = csp.tile([128, KC, BL], f32, tag="dv")
                        nc.vector.tensor_sub(dv[:], opv[:], pblk[:, s, :, :])
                        ev = csp.tile([128, KC, BL], f32, tag="ev")
                        nc.vector.tensor_mul(ev[:], hg[:], dv[:])
                        nc.vector.tensor_add(ring[:, s, :, :], ev[:],
                                             pblk[:, s, :, :])
                        if cut_chain:
                            u_nxt = cup.tile([128, KC, BL], bf16, tag="u")
                        elif s == R - 1:
                            u_nxt = u_st
                        else:
                            u_nxt = cup.tile([128, KC, BL], bf16, tag="u")
                        nc.vector.tensor_add(u_nxt[:], xp[:, s, :, :], ev[:])
                        u_cur = u_st if cut_chain else u_nxt

                    nc.sync.dma_start(outT[:, ds(i * R * CW, R * CW)], ring[:])

    nc.compile()
    return nc


def _get_module(T, R, iters=None):
    key = (T, R, iters)
    if key not in _CACHE:
        _CACHE[key] = _build(T, R, iters)
    return _CACHE[key]


def _make_in_maps(x, W_in, b_in, R):
    B, T, D = x.shape
    Wt = W_in.T  # [D, 6H]
    W5 = Wt[:, :NG * H]
    W6 = Wt[:, NG * H:]
    w5_arr = np.ascontiguousarray(
        W5.reshape(KC, 128, M5, 128).transpose(1, 2, 0, 3)
        .reshape(128, M5 * KC * 128)).astype(ml_dtypes.bfloat16)
    w6_arr = np.ascontiguousarray(
        W6.reshape(KC, 128, KC, 128).transpose(1, 2, 0, 3)
        .reshape(128, KC * KC * 128)).astype(np.float32)
    b5k_arr = np.ascontiguousarray((2.0 * b_in[:NG * H]).reshape(M5, 128)
                                   ).astype(ml_dtypes.bfloat16)
    sel_arr = np.zeros((KC, CW), ml_dtypes.bfloat16)
    for k in range(KC):
        sel_arr[k, BL * k:BL * k + BL] = 1.0
    b6_arr = np.ascontiguousarray(b_in[NG * H:].reshape(KC, 128).T
                                  ).astype(np.float32)
    in_maps = []
    for c in range(NCORES):
        xs = x[BL * c:BL * (c + 1)]  # [BL, T, D]
        xTa = np.zeros((128, (T + R) * CW), np.float32)
        xTa[:, :T * CW] = (xs.reshape(BL, T, KC, 128).transpose(3, 1, 2, 0)
                           .reshape(128, T * CW))
        in_maps.append({"xT": xTa, "w5": w5_arr, "w6": w6_arr,
                        "b5k": b5k_arr, "sel": sel_arr, "b6": b6_arr})
    return in_maps


def kernel(x, lengths, W_in, b_in):
    from concourse import bass_utils

    x = np.asarray(x, dtype=np.float32)
    lengths = np.asarray(lengths).astype(np.int64)
    W_in = np.asarray(W_in, dtype=np.float32)
    b_in = np.asarray(b_in, dtype=np.float32)
    B, T, D = x.shape
    R = 32
    nc = _get_module(T, R)
    in_maps = _make_in_maps(x, W_in, b_in, R)
    res = bass_utils.run_bass_kernel_spmd(nc, in_maps,
                                          core_ids=list(range(NCORES)))
    out = np.zeros((B, T, D), np.float32)
    for c in range(NCORES):
        oT = np.asarray(res.results[c]["outT"])
        oc = (oT.reshape(128, T, KC, BL).transpose(3, 1, 2, 0)
              .reshape(BL, T, D))
        out[BL * c:BL * (c + 1)] = oc
    mask = np.arange(T)[None, :] < lengths[:, None]
    out *= mask[:, :, None].astype(np.float32)
    return out
